# revision 9
# baseline (speedup 1.0000x reference)
"""MixIT loss kernel for Trainium2 (8 NeuronCores, Bass/Tile).

Math: reference computes, for each of 16 assignment combinations k,
    mix[k,b,c,t] = sum_s A[k,c,s] * x[b,s,t]        (A tiny [16,2,4])
    loss[k] = sum_b [ snr(mix[k,b,0], m1[b]) + snr(mix[k,b,1], m2[b]) ]
    snr(y, m) = 10*log10(sum_t (y-m)^2 + 30*sum_t y^2) - 10*log10(sum_t y^2)
and returns (argmin_k, min_k).

Since mix is linear in x, every sum over T is a quadratic form in the Gram
matrix of the per-batch streams {x_0..x_3, m1, m2} over T=64000.  The device
only computes pairwise dot products; the 16-combination argmin/min
(O(16*32) flops) is finished on host.

Device layout per core (4 batches = 24 streams, interleaved host-side into
one [4, 6, T] tensor so each chunk needs only two DMA configs):
T is split as 128 partitions x 500 cols.  Cols are chunked (185, 185, 130)
so every DMA run is >= 512B (below that the DMA engines run at half rate).
Per chunk: DMA lands stream-major zA[128, 24, cq] f32, then DVE + Act +
GpSimd re-layout (and cast) to f-major bf16 zB[128, cq, 24], and the PE
runs cq/5 accumulating bf16 matmuls with lhsT = rhs = zB[:, 5g:5g+5, :]
— a contiguous [128, 120] operand (24 streams x 5 T-cols), amortizing the
fixed LDWEIGHTS cost (the BIR verifier requires a 2D stationary operand,
hence the f-major re-layout).  bf16 runs the PE at 1 cycle/row vs fp32's 4;
the min-vs-2nd-best combo gap (1.3e-3 rel) is ~30x the bf16-induced loss
perturbation (~5e-5), so argmin is stable.
out[120,120] accumulates in PSUM f32; entries with mismatched T-col are
junk, and the host sums the 5 aligned diagonal [24,24] blocks:
G[j,k] = sum_f out[24f+j, 24f+k].
"""

import itertools
import sys

import numpy as np

if "/opt/trn_rl_repo" not in sys.path:
    sys.path.insert(0, "/opt/trn_rl_repo")

N_CORES = 8
B = 32               # full batch
S = 4                # estimated sources
T = 64000
BL = B // N_CORES    # batches per core = 4
NJ = 6 * BL          # streams per core = 24 (per batch: 4 x, m1, m2)
P = 128
COLS = T // P        # 500
FG = 5               # T-cols fused per matmul (5*24 = 120-col operands)
# Col chunks: every (partition, stream) DMA run is cq*4 bytes; runs < 512B
# halve DMA throughput, so all chunks are >= 128 cols.  Divisible by 5 so
# matmul groups never straddle chunks.  Last chunk smallest: it is the only
# one whose transpose + matmuls sit past the end of the DMA stream.
CHUNKS = (185, 185, 130)
NQ = len(CHUNKS)
assert sum(CHUNKS) == COLS and all(c % FG == 0 and c >= 128 for c in CHUNKS)
# Transpose engine split (fractions of each chunk's cols): measured rates
# are DVE ~2.0 ns/elem, Act ~2.2, GpSimd ~3.5; split inversely.
TSPLIT = (("vector", 0.41), ("scalar", 0.36), ("gpsimd", 0.23))
TSLICE = 28          # max cols per transpose copy instruction
SNR_MAX = 30.0

_CACHE = {}
LAST_RESULTS = None  # BassKernelResults of the most recent run (for test harness)


def _engine_cols(cq):
    """Split cq cols into per-engine contiguous ranges (multiples of FG)."""
    cuts = []
    acc = 0
    for _, frac in TSPLIT[:-1]:
        acc += frac
        cuts.append(int(round(cq * acc / FG)) * FG)
    bounds = [0] + cuts + [cq]
    return [(TSPLIT[i][0], bounds[i], bounds[i + 1]) for i in range(len(TSPLIT))]


# Measured cast cost per col (24 streams) on each engine, ns.
_CAST_NS_PER_COL = {"vector": 48.0, "scalar": 52.0, "gpsimd": 87.0}


def _chunk_plan(cq):
    """Cast slices + matmul emission order for one chunk.

    Slices are even 5-col-multiple splits of each engine's range.  Matmul
    groups are ordered by the estimated finish time of the cast slice that
    covers them: the PE runs in-order, so emitting groups in data-arrival
    order keeps it from stalling on one late slice while later-emitted
    groups already have data.
    """
    slices = []          # (engine, s0, w)
    finish = [0.0] * cq  # per-col estimated cast finish (ns from chunk start)
    for ename, e0, e1 in _engine_cols(cq):
        span = e1 - e0
        if span <= 0:
            continue
        nsl = max(1, -(-span // TSLICE))
        base = span // nsl // FG * FG
        widths = [base] * nsl
        extra = span - base * nsl
        i = 0
        while extra > 0:
            widths[i % nsl] += FG
            extra -= FG
            i += 1
        t = 0.0
        s0 = e0
        for w in widths:
            t += w * _CAST_NS_PER_COL[ename]
            slices.append((ename, s0, w))
            for c in range(s0, s0 + w):
                finish[c] = t
            s0 += w
    ngroups = cq // FG
    gfin = [max(finish[FG * i:FG * i + FG]) for i in range(ngroups)]
    order = sorted(range(ngroups), key=lambda i: (gfin[i], i))
    return slices, order


def _build_nc():
    from concourse import bacc, bass, tile
    import concourse.mybir as mybir

    nc = bacc.Bacc("TRN2", target_bir_lowering=False, debug=False,
                   num_devices=N_CORES)
    f32 = mybir.dt.float32
    bf16 = mybir.dt.bfloat16
    z = nc.dram_tensor("z", [BL, 6, T], f32, kind="ExternalInput")
    g = nc.dram_tensor("g", [2, NJ * FG, NJ * FG], f32, kind="ExternalOutput")

    with tile.TileContext(nc) as tc:
        with (
            tc.tile_pool(name="za", bufs=NQ) as zapool,
            tc.tile_pool(name="zb", bufs=NQ) as zbpool,
            tc.tile_pool(name="ps", bufs=1, space=bass.MemorySpace.PSUM) as psp,
            tc.tile_pool(name="o", bufs=1) as opool,
        ):
            acc_a = psp.tile([NJ * FG, NJ * FG], f32, tag="pa")
            acc_b = psp.tile([NJ * FG, NJ * FG], f32, tag="pb")

            # All input DMAs up front: each chunk split over both HWDGE
            # rings (sync & scalar) so descriptor generation is parallel.
            # za bufs=NQ, so every config can fire immediately and the
            # rings stream chunk after chunk with no dependency stalls.
            zr = z.ap().rearrange("b s (p c) -> p (b s) c", p=P)
            zas = []
            c0 = 0
            for cq in CHUNKS:
                za = zapool.tile([P, NJ, max(CHUNKS)], f32, tag="za")
                nc.sync.dma_start(
                    out=za[:, 0:NJ // 2, 0:cq],
                    in_=zr[:, 0:NJ // 2, c0:c0 + cq],
                )
                nc.scalar.dma_start(
                    out=za[:, NJ // 2:NJ, 0:cq],
                    in_=zr[:, NJ // 2:NJ, c0:c0 + cq],
                )
                zas.append(za)
                c0 += cq

            engines = {"vector": nc.vector, "scalar": nc.scalar,
                       "gpsimd": nc.gpsimd}
            for q, cq in enumerate(CHUNKS):
                # Pin scheduler order: the list scheduler's cost model badly
                # underestimates real DMA time and will otherwise hoist a
                # later chunk's matmul ahead of earlier chunks' stragglers
                # in the in-order PE stream, serializing the whole pipeline
                # behind one long semaphore wait (measured: a 9us PE stall).
                # The floor is a scheduler-sim timestamp only; hardware
                # still runs purely on semaphores.
                tc.tile_set_cur_wait(q * 0.012)
                za = zas[q]
                zb = zbpool.tile([P, max(CHUNKS), NJ], bf16, tag="zb")
                # f-major re-layout + f32->bf16 cast, split across three
                # otherwise-idle engines working disjoint col ranges.
                slices, order = _chunk_plan(cq)
                for ename, s0, w in slices:
                    eng = engines[ename]
                    cp_dst = zb[:, s0:s0 + w, :]
                    cp_src = za[:, :, s0:s0 + w].transpose([0, 2, 1])
                    if ename == "scalar":
                        eng.copy(cp_dst, cp_src)
                    else:
                        eng.tensor_copy(cp_dst, cp_src)
                acc = acc_b if q == NQ - 1 else acc_a
                for n, i in enumerate(order):
                    op = zb[:, FG * i:FG * (i + 1), :]
                    nc.tensor.matmul(
                        acc[:, :], op, op,
                        start=(n == 0 and q in (0, NQ - 1)),
                        stop=(n == len(order) - 1 and q in (NQ - 2, NQ - 1)),
                    )
            # Drains AFTER all cast emission so their semaphore waits (on the
            # banks' final matmuls) never block a cast engine's queue.  Bank A
            # (chunks 0-1) drains via Scalar while the PE runs chunk 2; bank B
            # via Vector at the end.  DMA cannot read PSUM, so bounce via SBUF.
            tc.tile_set_cur_wait(NQ * 0.012)
            gout_a = opool.tile([NJ * FG, NJ * FG], f32, tag="oa")
            nc.scalar.copy(gout_a[:, :], acc_a[:, :])
            nc.sync.dma_start(out=g.ap()[0], in_=gout_a[:, :])
            gout_b = opool.tile([NJ * FG, NJ * FG], f32, tag="ob")
            nc.vector.tensor_copy(gout_b[:, :], acc_b[:, :])
            nc.sync.dma_start(out=g.ap()[1], in_=gout_b[:, :])
    nc.compile()
    return nc


def _get_nc():
    if "nc" not in _CACHE:
        _CACHE["nc"] = _build_nc()
    return _CACHE["nc"]


def _finish_host(grams: np.ndarray):
    """grams: [N_CORES, 2, 120, 120] per-core PE blocks -> (argmin, min)."""
    # Collapse the fused T-col axis: G[j,k] = sum_f out[24f+j, 24f+k].
    g5 = grams.reshape(N_CORES, 2, FG, NJ, FG, NJ).astype(np.float64)
    g24 = np.einsum("cafjfk->cjk", g5)

    # Per full-batch index b: core c = b // BL, local l = b % BL.
    # Stream layout per core: x_(l,s) at 6*l+s, m1_l at 6*l+4, m2_l at 6*l+5.
    Gxx = np.empty((B, S, S), np.float64)   # sum_t x_s x_s'
    C1 = np.empty((B, S), np.float64)       # sum_t x_s m1
    C2 = np.empty((B, S), np.float64)
    M1 = np.empty((B,), np.float64)         # sum_t m1^2
    M2 = np.empty((B,), np.float64)
    for b in range(B):
        c, l = divmod(b, BL)
        gm = g24[c]
        xs = slice(6 * l, 6 * l + S)
        Gxx[b] = gm[xs, xs]
        C1[b] = gm[xs, 6 * l + 4]
        C2[b] = gm[xs, 6 * l + 5]
        M1[b] = gm[6 * l + 4, 6 * l + 4]
        M2[b] = gm[6 * l + 5, 6 * l + 5]

    combos = np.array(list(itertools.product([0, 1], repeat=S)), np.float64)
    losses = np.zeros(len(combos), np.float64)
    with np.errstate(divide="ignore"):
        for w, cc, mm in ((combos, C1, M1), (1.0 - combos, C2, M2)):
            bq = np.einsum("ks,bst,kt->kb", w, Gxx, w)        # sum_t y^2
            aq = bq - 2.0 * (w @ cc.T) + mm[None, :]          # sum_t (y-m)^2
            losses += np.sum(10.0 * np.log10(aq + SNR_MAX * bq)
                             - 10.0 * np.log10(bq), axis=1)
    k = int(np.argmin(losses))
    return np.int32(k), np.float32(losses[k])


def _ensure_trace_hook_safe():
    """If BASS_TRACE is set but this image lacks antenv.axon_hooks, install a
    null hook module so run_bass_kernel_spmd degrades to an untraced run
    instead of crashing on the import."""
    try:
        import antenv.axon_hooks  # noqa: F401
    except ImportError:
        import types

        stub = types.ModuleType("antenv.axon_hooks")
        stub.get_axon_ntff_profile_hook = lambda: None
        stub.set_axon_ntff_profile_hook = lambda h: None
        sys.modules["antenv.axon_hooks"] = stub


def kernel(estimated_sources: np.ndarray, m1: np.ndarray, m2: np.ndarray):
    global LAST_RESULTS
    _ensure_trace_hook_safe()
    from concourse.bass_utils import run_bass_kernel_spmd

    x = np.asarray(estimated_sources, dtype=np.float32)
    m1 = np.asarray(m1, dtype=np.float32)
    m2 = np.asarray(m2, dtype=np.float32)

    in_maps = []
    for c in range(N_CORES):
        sl = slice(BL * c, BL * (c + 1))
        z = np.empty((BL, 6, T), np.float32)
        z[:, 0:S] = x[sl]
        z[:, S] = m1[sl]
        z[:, S + 1] = m2[sl]
        in_maps.append({"z": z})

    nc = _get_nc()
    LAST_RESULTS = run_bass_kernel_spmd(nc, in_maps, list(range(N_CORES)))
    grams = np.stack([LAST_RESULTS.results[c]["g"] for c in range(N_CORES)])
    return _finish_host(grams)
